# revision 4
# baseline (speedup 1.0000x reference)
"""GCN-with-edge-features kernel for 8 Trainium2 cores.

Strategy (per sharding hint): data-parallel over edges. The dominant
compute is the two edge-net MLPs producing per-edge weight matrices
  theta1 = relu(ea @ W1a + b1a) @ W1b + b1b   [E, 1024]
  theta2 = relu(ea @ W2a + b2a) @ W2b + b2b   [E, 2048]
(~630 GFLOP total, bf16 on the tensor engine), FUSED on-device with the
per-edge matmul msg[e,:] = x_src[e,:] @ theta[e].reshape(FI, FO) so the
huge theta intermediate never leaves PSUM/SBUF. Layout: edges on the
PSUM partition dim, theta[e, d] with d free; the i-contraction runs on
the vector engine as 32 fused multiply-adds with the per-edge scalar
x_src[e, i] as the per-partition scalar operand. The theta bias folds
into the accumulator init: acc0 = x_src @ B, B[i, o] = b[i*FO+o].

Two launches (layer 1, layer 2) with the segment-mean node aggregation
on host in between; graph pooling + FC head also on host (trivial).
12500 edges per core, padded to 12800 = 25 chunks of 512.
"""
import numpy as np

import sys
for p in ("/opt/trn_rl_repo",):
    if p not in sys.path:
        sys.path.append(p)

import ml_dtypes

from concourse import bass, bacc, mybir, tile
from concourse import bass_utils

E = 100000
N = 50000
NG = 2000
F_IN = 32
EF = 16
H = 32
H2 = 64
NC = 8
EPC = E // NC          # 12500 edges per core
CH = 512
NCHUNK = 25
EP = CH * NCHUNK       # 12800 padded edges per core
NT = EP // 128         # 100 e-tiles of 128 edges
D1 = H * F_IN          # 1024
D2 = H * H2            # 2048

_F32 = mybir.dt.float32
_BF16 = mybir.dt.bfloat16
_RELU = mybir.ActivationFunctionType.Relu
_COPY = mybir.ActivationFunctionType.Copy
_MUL = mybir.AluOpType.mult
_ADD = mybir.AluOpType.add

_NC_CACHE = {}
LAST_RUNS = []  # BassKernelResults of the device launches in the last kernel() call

BF16 = ml_dtypes.bfloat16


def _build_pass(fo):
    """One GNN layer: fo = per-edge output width (32 for layer 1, 64 for
    layer 2). The per-edge input width is 32 for both layers."""
    fi = 32
    d = fi * fo                    # theta width: 1024 / 2048
    nb = d // 512                  # PSUM banks per e-tile: 2 / 4

    nc = bacc.Bacc(None, target_bir_lowering=False)

    eaT_d = nc.dram_tensor("eaT", [EF, EP], _BF16, kind="ExternalInput")
    Wa_d = nc.dram_tensor("Wa", [EF, D1], _BF16, kind="ExternalInput")
    Wb_d = nc.dram_tensor("Wb", [128, 8 * d], _BF16, kind="ExternalInput")
    ba_d = nc.dram_tensor("ba", [128, 8], _F32, kind="ExternalInput")
    Bb_d = nc.dram_tensor("Bb", [fi, fo], _BF16, kind="ExternalInput")
    xsS_d = nc.dram_tensor("xsS", [128, NT * fi], _F32, kind="ExternalInput")
    xsT_d = nc.dram_tensor("xsT", [fi, EP], _BF16, kind="ExternalInput")
    msg_d = nc.dram_tensor("msg", [128, NT * fo], _F32, kind="ExternalOutput")

    with tile.TileContext(nc) as tc:
        with (
            tc.tile_pool(name="w", bufs=1) as wpool,
            tc.tile_pool(name="h", bufs=2) as hpool,
            tc.tile_pool(name="psa", bufs=2, space=bass.MemorySpace.PSUM) as psapool,
            tc.tile_pool(name="th", bufs=4, space=bass.MemorySpace.PSUM) as thpool,
            tc.tile_pool(name="psb", bufs=2, space=bass.MemorySpace.PSUM) as psbpool,
        ):
            wa = wpool.tile([EF, D1], _BF16)
            nc.sync.dma_start(wa[:], Wa_d[:])
            wb = wpool.tile([128, 8 * d], _BF16)
            nc.sync.dma_start(wb[:], Wb_d[:])
            ba = wpool.tile([128, 8], _F32)
            nc.sync.dma_start(ba[:], ba_d[:])
            bb = wpool.tile([fi, fo], _BF16)
            nc.sync.dma_start(bb[:], Bb_d[:])
            ea = wpool.tile([EF, EP], _BF16)
            nc.sync.dma_start(ea[:], eaT_d[:])
            xss = wpool.tile([128, NT * fi], _F32)
            nc.sync.dma_start(xss[:], xsS_d[:])
            xst = wpool.tile([fi, EP], _BF16)
            nc.sync.dma_start(xst[:], xsT_d[:])
            msg = wpool.tile([128, NT * fo], _F32)

            for c in range(NCHUNK):
                # stage A: h = relu(ea @ Wa + ba), k on partitions
                h = hpool.tile([128, 8 * CH], _BF16)
                for j in range(8):
                    psa = psapool.tile([128, CH], _F32)
                    nc.tensor.matmul(
                        psa[:], wa[:, j * 128:(j + 1) * 128],
                        ea[:, c * CH:(c + 1) * CH], start=True, stop=True,
                    )
                    nc.scalar.activation(
                        h[:, j * CH:(j + 1) * CH], psa[:], _RELU,
                        bias=ba[:, j:j + 1],
                    )

                for t in range(4):
                    tg = c * 4 + t            # global e-tile id
                    e0 = t * 128              # edge offset within chunk
                    # accumulator init: acc0 = xs @ B (theta-bias fold)
                    psb = psbpool.tile([128, fo], _F32)
                    nc.tensor.matmul(
                        psb[:], xst[:, tg * 128:(tg + 1) * 128], bb[:],
                        start=True, stop=True,
                    )
                    mslice = msg[:, tg * fo:(tg + 1) * fo]
                    nc.scalar.activation(mslice, psb[:], _COPY)

                    # stage B: theta tile [128 edges, d], bank-staggered
                    th = [thpool.tile([128, 512], _F32) for _ in range(nb)]
                    for b in range(nb):
                        for k in range(8):
                            nc.tensor.matmul(
                                th[b][:],
                                h[:, k * CH + e0:k * CH + e0 + 128],
                                wb[:, k * d + b * 512:k * d + (b + 1) * 512],
                                start=(k == 0), stop=(k == 7),
                            )

                    # per-edge contraction: msg += xs[:, i] * theta[:, i*fo:+fo]
                    npb = 512 // fo           # i-values per bank: 16 / 8
                    for i in range(fi):
                        nc.vector.scalar_tensor_tensor(
                            mslice,
                            th[i // npb][:, (i % npb) * fo:(i % npb + 1) * fo],
                            xss[:, tg * fi + i:tg * fi + i + 1],
                            mslice,
                            _MUL, _ADD,
                        )

            nc.sync.dma_start(msg_d[:], msg[:])

    nc.compile()
    return nc


def _get_nc(fo):
    if fo not in _NC_CACHE:
        _NC_CACHE[fo] = _build_pass(fo)
    return _NC_CACHE[fo]


def _relu(v):
    return np.maximum(v, 0.0)


class _SegMean:
    """Sort-based segment mean (np.add.at is too slow)."""

    def __init__(self, idx, n):
        self.n = n
        self.order = np.argsort(idx, kind="stable")
        sorted_idx = np.asarray(idx)[self.order]
        self.uniq, self.starts = np.unique(sorted_idx, return_index=True)
        self.cnt = np.maximum(
            np.bincount(np.asarray(idx), minlength=n), 1.0
        ).astype(np.float32)[:, None]

    def __call__(self, vals):
        out = np.zeros((self.n, vals.shape[1]), np.float32)
        out[self.uniq] = np.add.reduceat(vals[self.order], self.starts, axis=0)
        return out / self.cnt


def _pack_xs(xs_full, fi):
    """[E, fi] fp32 -> per-core ([128, NT*fi] fp32 tiled, [fi, EP] bf16 T)."""
    outs = []
    for i in range(NC):
        sh = np.zeros((EP, fi), np.float32)
        sh[:EPC] = xs_full[i * EPC:(i + 1) * EPC]
        tiled = np.ascontiguousarray(
            sh.reshape(NT, 128, fi).transpose(1, 0, 2).reshape(128, NT * fi))
        tr = np.ascontiguousarray(sh.T.astype(BF16))
        outs.append((tiled, tr))
    return outs


def _unpack_msg(results, fo):
    """per-core [128, NT*fo] -> [E, fo]."""
    parts = []
    for i in range(NC):
        m = results[i]["msg"]
        parts.append(
            m.reshape(128, NT, fo).transpose(1, 0, 2).reshape(EP, fo)[:EPC])
    return np.concatenate(parts, axis=0)


def _run_pass(fo, ea_bf_cores, xs_full, Wa, Wb, ba, Bb):
    nc = _get_nc(fo)
    wb_packed = np.ascontiguousarray(
        np.concatenate([Wb[j * 128:(j + 1) * 128, :] for j in range(8)], axis=1)
    ).astype(BF16)
    wa_bf = np.ascontiguousarray(Wa).astype(BF16)
    ba_t = np.ascontiguousarray(ba.reshape(8, 128).T)
    bb_r = np.ascontiguousarray(Bb.reshape(32, fo)).astype(BF16)
    xs_packed = _pack_xs(xs_full, 32)
    in_maps = []
    for i in range(NC):
        in_maps.append(dict(
            eaT=ea_bf_cores[i], Wa=wa_bf, Wb=wb_packed, ba=ba_t, Bb=bb_r,
            xsS=xs_packed[i][0], xsT=xs_packed[i][1],
        ))
    res = bass_utils.run_bass_kernel_spmd(nc, in_maps, core_ids=list(range(NC)))
    LAST_RUNS.append(res)
    return _unpack_msg(res.results, fo)


def kernel(**inputs):
    x = np.asarray(inputs["x"], np.float32)
    edge_index = np.asarray(inputs["edge_index"])
    eap = np.asarray(inputs["edge_attr_packed"])
    batch = np.asarray(inputs["batch"])
    W1a = np.asarray(inputs["W1a"], np.float32)
    W1b = np.asarray(inputs["W1b"], np.float32)
    W2a = np.asarray(inputs["W2a"], np.float32)
    W2b = np.asarray(inputs["W2b"], np.float32)
    b1a = np.asarray(inputs["b1a"], np.float32)
    b1b = np.asarray(inputs["b1b"], np.float32)
    b2a = np.asarray(inputs["b2a"], np.float32)
    b2b = np.asarray(inputs["b2b"], np.float32)
    root1 = np.asarray(inputs["root1"], np.float32)
    bias1 = np.asarray(inputs["bias1"], np.float32)
    root2 = np.asarray(inputs["root2"], np.float32)
    bias2 = np.asarray(inputs["bias2"], np.float32)

    LAST_RUNS.clear()

    # MSB-first bit unpack -> [E, 16], per-core transposed bf16 copies
    shifts = np.arange(7, -1, -1, dtype=np.int32)
    ea = ((eap[:, :, None].astype(np.int32) >> shifts) & 1).reshape(E, -1)
    ea = ea.astype(np.float32)
    ea_cores = []
    for i in range(NC):
        sh = np.zeros((EP, EF), np.float32)
        sh[:EPC] = ea[i * EPC:(i + 1) * EPC]
        ea_cores.append(np.ascontiguousarray(sh.T.astype(BF16)))

    src, dst = edge_index[0], edge_index[1]
    segmean_dst = _SegMean(dst, N)

    msg1 = _run_pass(32, ea_cores, x[src], W1a, W1b, b1a, b1b)
    h = _relu(segmean_dst(msg1) + x @ root1 + bias1)

    msg2 = _run_pass(64, ea_cores, h[src], W2a, W2b, b2a, b2b)
    h = _relu(segmean_dst(msg2) + h @ root2 + bias2)

    g = _SegMean(batch, NG)(h)
    g = _relu(g @ np.asarray(inputs["fcW1"], np.float32) + np.asarray(inputs["fcb1"], np.float32))
    g = _relu(g @ np.asarray(inputs["fcW2"], np.float32) + np.asarray(inputs["fcb2"], np.float32))
    g = _relu(g @ np.asarray(inputs["fcW3"], np.float32) + np.asarray(inputs["fcb3"], np.float32))
    return (g @ np.asarray(inputs["fcW4"], np.float32) + np.asarray(inputs["fcb4"], np.float32)).astype(np.float32)


# revision 6
# speedup vs baseline: 4.0596x; 4.0596x over previous
"""GCN-with-edge-features kernel for 8 Trainium2 cores.

Strategy (per sharding hint): data-parallel over edges. The dominant
compute is the two edge-net MLPs producing per-edge weight matrices
  theta1 = relu(ea @ W1a + b1a) @ W1b + b1b   [E, 1024]
  theta2 = relu(ea @ W2a + b2a) @ W2b + b2b   [E, 2048]
(~630 GFLOP total, bf16 on the tensor engine), FUSED on-device with the
per-edge matmul msg[e,:] = x_src[e,:] @ theta[e].reshape(FI, FO) so the
huge theta intermediate never leaves PSUM/SBUF. Layout: edges on the
PSUM partition dim, theta[e, d] with d free; the i-contraction runs on
the vector engine as 32 fused multiply-adds with the per-edge scalar
x_src[e, i] as the per-partition scalar operand. The theta bias folds
into the accumulator init: acc0 = x_src @ B, B[i, o] = b[i*FO+o].

Two launches (layer 1, layer 2) with the segment-mean node aggregation
on host in between; graph pooling + FC head also on host (trivial).
12500 edges per core, padded to 12800 = 25 chunks of 512.
"""
import numpy as np

import sys
for p in ("/opt/trn_rl_repo",):
    if p not in sys.path:
        sys.path.append(p)

import ml_dtypes

from concourse import bass, bacc, mybir, tile
from concourse import bass_utils

E = 100000
N = 50000
NG = 2000
F_IN = 32
EF = 16
H = 32
H2 = 64
NC = 8
EPC = E // NC          # 12500 edges per core
CH = 512
NCHUNK = 25
EP = CH * NCHUNK       # 12800 padded edges per core
NT = EP // 128         # 100 e-tiles of 128 edges
D1 = H * F_IN          # 1024
D2 = H * H2            # 2048

_F32 = mybir.dt.float32
_BF16 = mybir.dt.bfloat16
_RELU = mybir.ActivationFunctionType.Relu
_COPY = mybir.ActivationFunctionType.Copy
_MUL = mybir.AluOpType.mult
_ADD = mybir.AluOpType.add

_NC_CACHE = {}
LAST_RUNS = []  # BassKernelResults of the device launches in the last kernel() call

BF16 = ml_dtypes.bfloat16


def _build_pass(fo):
    """One GNN layer: fo = per-edge output width (32 for layer 1, 64 for
    layer 2). The per-edge input width is 32 for both layers."""
    fi = 32
    d = fi * fo                    # theta width: 1024 / 2048
    nb = d // 512                  # PSUM banks per e-tile: 2 / 4

    nc = bacc.Bacc(None, target_bir_lowering=False)

    eaT_d = nc.dram_tensor("eaT", [EF, EP], _BF16, kind="ExternalInput")
    Wa_d = nc.dram_tensor("Wa", [EF, D1], _BF16, kind="ExternalInput")
    Wb_d = nc.dram_tensor("Wb", [128, 8 * d], _BF16, kind="ExternalInput")
    ba_d = nc.dram_tensor("ba", [128, 8], _F32, kind="ExternalInput")
    Bb_d = nc.dram_tensor("Bb", [fi, fo], _BF16, kind="ExternalInput")
    xsS_d = nc.dram_tensor("xsS", [128, NT * fi], _F32, kind="ExternalInput")
    xsT_d = nc.dram_tensor("xsT", [fi, EP], _BF16, kind="ExternalInput")
    msg_d = nc.dram_tensor("msg", [128, NT * fo], _F32, kind="ExternalOutput")

    with tile.TileContext(nc) as tc:
        with (
            tc.tile_pool(name="w", bufs=1) as wpool,
            tc.tile_pool(name="h", bufs=2) as hpool,
            tc.tile_pool(name="psa", bufs=2, space=bass.MemorySpace.PSUM) as psapool,
            tc.tile_pool(name="th", bufs=4, space=bass.MemorySpace.PSUM) as thpool,
            tc.tile_pool(name="psb", bufs=2, space=bass.MemorySpace.PSUM) as psbpool,
        ):
            wa = wpool.tile([EF, D1], _BF16)
            nc.sync.dma_start(wa[:], Wa_d[:])
            wb = wpool.tile([128, 8 * d], _BF16)
            nc.sync.dma_start(wb[:], Wb_d[:])
            ba = wpool.tile([128, 8], _F32)
            nc.sync.dma_start(ba[:], ba_d[:])
            bb = wpool.tile([fi, fo], _BF16)
            nc.sync.dma_start(bb[:], Bb_d[:])
            ea = wpool.tile([EF, EP], _BF16)
            nc.sync.dma_start(ea[:], eaT_d[:])
            xss = wpool.tile([128, NT * fi], _F32)
            nc.sync.dma_start(xss[:], xsS_d[:])
            xst = wpool.tile([fi, EP], _BF16)
            nc.sync.dma_start(xst[:], xsT_d[:])
            msg = wpool.tile([128, NT * fo], _F32)

            for c in range(NCHUNK):
                # stage A: h = relu(ea @ Wa + ba), k on partitions
                h = hpool.tile([128, 8 * CH], _BF16)
                for j in range(8):
                    psa = psapool.tile([128, CH], _F32)
                    nc.tensor.matmul(
                        psa[:], wa[:, j * 128:(j + 1) * 128],
                        ea[:, c * CH:(c + 1) * CH], start=True, stop=True,
                    )
                    nc.scalar.activation(
                        h[:, j * CH:(j + 1) * CH], psa[:], _RELU,
                        bias=ba[:, j:j + 1],
                    )

                for t in range(4):
                    tg = c * 4 + t            # global e-tile id
                    e0 = t * 128              # edge offset within chunk
                    # accumulator init: acc0 = xs @ B (theta-bias fold)
                    psb = psbpool.tile([128, fo], _F32)
                    nc.tensor.matmul(
                        psb[:], xst[:, tg * 128:(tg + 1) * 128], bb[:],
                        start=True, stop=True,
                    )
                    mslice = msg[:, tg * fo:(tg + 1) * fo]
                    nc.scalar.activation(mslice, psb[:], _COPY)

                    # stage B: theta tile [128 edges, d], bank-staggered
                    th = [thpool.tile([128, 512], _F32, name="th", tag="th")
                          for _ in range(nb)]
                    for b in range(nb):
                        for k in range(8):
                            nc.tensor.matmul(
                                th[b][:],
                                h[:, k * CH + e0:k * CH + e0 + 128],
                                wb[:, k * d + b * 512:k * d + (b + 1) * 512],
                                start=(k == 0), stop=(k == 7),
                            )

                    # per-edge contraction: msg += xs[:, i] * theta[:, i*fo:+fo]
                    npb = 512 // fo           # i-values per bank: 16 / 8
                    for i in range(fi):
                        nc.vector.scalar_tensor_tensor(
                            mslice,
                            th[i // npb][:, (i % npb) * fo:(i % npb + 1) * fo],
                            xss[:, tg * fi + i:tg * fi + i + 1],
                            mslice,
                            _MUL, _ADD,
                        )

            nc.sync.dma_start(msg_d[:], msg[:])

    nc.compile()
    return nc


def _get_nc(fo):
    if fo not in _NC_CACHE:
        _NC_CACHE[fo] = _build_pass(fo)
    return _NC_CACHE[fo]


def _relu(v):
    return np.maximum(v, 0.0)


class _SegMean:
    """Sort-based segment mean (np.add.at is too slow)."""

    def __init__(self, idx, n):
        self.n = n
        self.order = np.argsort(idx, kind="stable")
        sorted_idx = np.asarray(idx)[self.order]
        self.uniq, self.starts = np.unique(sorted_idx, return_index=True)
        self.cnt = np.maximum(
            np.bincount(np.asarray(idx), minlength=n), 1.0
        ).astype(np.float32)[:, None]

    def __call__(self, vals):
        out = np.zeros((self.n, vals.shape[1]), np.float32)
        out[self.uniq] = np.add.reduceat(vals[self.order], self.starts, axis=0)
        return out / self.cnt


def _pack_xs(xs_full, fi):
    """[E, fi] fp32 -> per-core ([128, NT*fi] fp32 tiled, [fi, EP] bf16 T)."""
    outs = []
    for i in range(NC):
        sh = np.zeros((EP, fi), np.float32)
        sh[:EPC] = xs_full[i * EPC:(i + 1) * EPC]
        tiled = np.ascontiguousarray(
            sh.reshape(NT, 128, fi).transpose(1, 0, 2).reshape(128, NT * fi))
        tr = np.ascontiguousarray(sh.T.astype(BF16))
        outs.append((tiled, tr))
    return outs


def _unpack_msg(results, fo):
    """per-core [128, NT*fo] -> [E, fo]."""
    parts = []
    for i in range(NC):
        m = results[i]["msg"]
        parts.append(
            m.reshape(128, NT, fo).transpose(1, 0, 2).reshape(EP, fo)[:EPC])
    return np.concatenate(parts, axis=0)


def _run_pass(fo, ea_bf_cores, xs_full, Wa, Wb, ba, Bb):
    nc = _get_nc(fo)
    wb_packed = np.ascontiguousarray(
        np.concatenate([Wb[j * 128:(j + 1) * 128, :] for j in range(8)], axis=1)
    ).astype(BF16)
    wa_bf = np.ascontiguousarray(Wa).astype(BF16)
    ba_t = np.ascontiguousarray(ba.reshape(8, 128).T)
    bb_r = np.ascontiguousarray(Bb.reshape(32, fo)).astype(BF16)
    xs_packed = _pack_xs(xs_full, 32)
    in_maps = []
    for i in range(NC):
        in_maps.append(dict(
            eaT=ea_bf_cores[i], Wa=wa_bf, Wb=wb_packed, ba=ba_t, Bb=bb_r,
            xsS=xs_packed[i][0], xsT=xs_packed[i][1],
        ))
    res = bass_utils.run_bass_kernel_spmd(nc, in_maps, core_ids=list(range(NC)))
    LAST_RUNS.append(res)
    return _unpack_msg(res.results, fo)


def kernel(**inputs):
    x = np.asarray(inputs["x"], np.float32)
    edge_index = np.asarray(inputs["edge_index"])
    eap = np.asarray(inputs["edge_attr_packed"])
    batch = np.asarray(inputs["batch"])
    W1a = np.asarray(inputs["W1a"], np.float32)
    W1b = np.asarray(inputs["W1b"], np.float32)
    W2a = np.asarray(inputs["W2a"], np.float32)
    W2b = np.asarray(inputs["W2b"], np.float32)
    b1a = np.asarray(inputs["b1a"], np.float32)
    b1b = np.asarray(inputs["b1b"], np.float32)
    b2a = np.asarray(inputs["b2a"], np.float32)
    b2b = np.asarray(inputs["b2b"], np.float32)
    root1 = np.asarray(inputs["root1"], np.float32)
    bias1 = np.asarray(inputs["bias1"], np.float32)
    root2 = np.asarray(inputs["root2"], np.float32)
    bias2 = np.asarray(inputs["bias2"], np.float32)

    LAST_RUNS.clear()

    # MSB-first bit unpack -> [E, 16], per-core transposed bf16 copies
    shifts = np.arange(7, -1, -1, dtype=np.int32)
    ea = ((eap[:, :, None].astype(np.int32) >> shifts) & 1).reshape(E, -1)
    ea = ea.astype(np.float32)
    ea_cores = []
    for i in range(NC):
        sh = np.zeros((EP, EF), np.float32)
        sh[:EPC] = ea[i * EPC:(i + 1) * EPC]
        ea_cores.append(np.ascontiguousarray(sh.T.astype(BF16)))

    src, dst = edge_index[0], edge_index[1]
    segmean_dst = _SegMean(dst, N)

    msg1 = _run_pass(32, ea_cores, x[src], W1a, W1b, b1a, b1b)
    h = _relu(segmean_dst(msg1) + x @ root1 + bias1)

    msg2 = _run_pass(64, ea_cores, h[src], W2a, W2b, b2a, b2b)
    h = _relu(segmean_dst(msg2) + h @ root2 + bias2)

    g = _SegMean(batch, NG)(h)
    g = _relu(g @ np.asarray(inputs["fcW1"], np.float32) + np.asarray(inputs["fcb1"], np.float32))
    g = _relu(g @ np.asarray(inputs["fcW2"], np.float32) + np.asarray(inputs["fcb2"], np.float32))
    g = _relu(g @ np.asarray(inputs["fcW3"], np.float32) + np.asarray(inputs["fcb3"], np.float32))
    return (g @ np.asarray(inputs["fcW4"], np.float32) + np.asarray(inputs["fcb4"], np.float32)).astype(np.float32)


# revision 7
# speedup vs baseline: 5.5172x; 1.3591x over previous
"""GCN-with-edge-features kernel for 8 Trainium2 cores.

Strategy (per sharding hint): data-parallel over edges. The dominant
compute is the two edge-net MLPs producing per-edge weight matrices
  theta1 = relu(ea @ W1a + b1a) @ W1b + b1b   [E, 1024]
  theta2 = relu(ea @ W2a + b2a) @ W2b + b2b   [E, 2048]
(~630 GFLOP total), FUSED on-device with the per-edge matmul
msg[e,:] = x_src[e,:] @ theta[e].reshape(FI, FO) so the huge theta
intermediate never leaves PSUM. Layout: edges on the PSUM partition
dim, theta[e, d] with d free. The big K=1024 GEMMs run in fp8e4 with
DoubleRow perf mode (2 k-planes per matmul); the small stage-A GEMM is
bf16. The i-contraction runs on the vector engine as one broadcasted
tensor_tensor multiply per PSUM bank plus a single fused reduction;
the theta bias b folds into the reduction via acc0 = x_src @ B,
B[i, o] = b[i*FO+o], computed by a tiny bf16 matmul.

Two launches (layer 1, layer 2) with the segment-mean node aggregation
on host in between; graph pooling + FC head also on host (trivial).
12500 edges per core, padded to 12800 = 25 chunks of 512.
"""
import numpy as np

import sys
for p in ("/opt/trn_rl_repo",):
    if p not in sys.path:
        sys.path.append(p)

import ml_dtypes

from concourse import bass, bacc, mybir, tile
from concourse import bass_utils

E = 100000
N = 50000
NG = 2000
F_IN = 32
EF = 16
H = 32
H2 = 64
NC = 8
EPC = E // NC          # 12500 edges per core
CH = 512
NCHUNK = 25
EP = CH * NCHUNK       # 12800 padded edges per core
NT = EP // 128         # 100 e-tiles of 128 edges
D1 = H * F_IN          # 1024
D2 = H * H2            # 2048

USE_FP8 = True         # fp8e4 + DoubleRow for the big stage-B GEMMs

_F32 = mybir.dt.float32
_BF16 = mybir.dt.bfloat16
_F8 = mybir.dt.float8e4
_RELU = mybir.ActivationFunctionType.Relu
_COPY = mybir.ActivationFunctionType.Copy
_MUL = mybir.AluOpType.mult
_ADD = mybir.AluOpType.add
_DR = mybir.MatmulPerfMode.DoubleRow

_NC_CACHE = {}
LAST_RUNS = []  # BassKernelResults of the device launches in the last kernel() call

BF16 = ml_dtypes.bfloat16
F8E4 = ml_dtypes.float8_e4m3fn


def _build_pass(fo):
    """One GNN layer: fo = per-edge output width (32 for layer 1, 64 for
    layer 2). The per-edge input width is 32 for both layers."""
    fi = 32
    d = fi * fo                    # theta width: 1024 / 2048
    nb = d // 512                  # PSUM banks per e-tile: 2 / 4
    npb = 512 // fo                # i-values per bank: 16 / 8
    hdt = _F8 if USE_FP8 else _BF16

    nc = bacc.Bacc(None, target_bir_lowering=False)

    eaT_d = nc.dram_tensor("eaT", [EF, EP], _BF16, kind="ExternalInput")
    Wa_d = nc.dram_tensor("Wa", [EF, D1], _BF16, kind="ExternalInput")
    Wb_d = nc.dram_tensor("Wb", [128, 8 * d], hdt, kind="ExternalInput")
    ba_d = nc.dram_tensor("ba", [128, 8], _F32, kind="ExternalInput")
    Bb_d = nc.dram_tensor("Bb", [fi, fo], _BF16, kind="ExternalInput")
    xsS_d = nc.dram_tensor("xsS", [128, NT * fi], _F32, kind="ExternalInput")
    xsT_d = nc.dram_tensor("xsT", [fi, EP], _BF16, kind="ExternalInput")
    msg_d = nc.dram_tensor("msg", [128, NT * fo], _F32, kind="ExternalOutput")

    with tile.TileContext(nc) as tc:
        with (
            tc.tile_pool(name="w", bufs=1) as wpool,
            tc.tile_pool(name="h", bufs=2) as hpool,
            tc.tile_pool(name="pr", bufs=3) as prpool,
            tc.tile_pool(name="psa", bufs=2, space=bass.MemorySpace.PSUM) as psapool,
            tc.tile_pool(name="th", bufs=4, space=bass.MemorySpace.PSUM) as thpool,
            tc.tile_pool(name="psb", bufs=2, space=bass.MemorySpace.PSUM) as psbpool,
        ):
            wa = wpool.tile([EF, D1], _BF16)
            nc.sync.dma_start(wa[:], Wa_d[:])
            wb = wpool.tile([128, 8, d], hdt)
            nc.sync.dma_start(wb[:].rearrange("p a b -> p (a b)"), Wb_d[:])
            ba = wpool.tile([128, 8], _F32)
            nc.sync.dma_start(ba[:], ba_d[:])
            bb = wpool.tile([fi, fo], _BF16)
            nc.sync.dma_start(bb[:], Bb_d[:])
            ea = wpool.tile([EF, EP], _BF16)
            nc.sync.dma_start(ea[:], eaT_d[:])
            xss = wpool.tile([128, NT * fi], _F32)
            nc.sync.dma_start(xss[:], xsS_d[:])
            xst = wpool.tile([fi, EP], _BF16)
            nc.sync.dma_start(xst[:], xsT_d[:])
            msg = wpool.tile([128, NT * fo], _F32)

            for c in range(NCHUNK):
                # stage A: h = relu(ea @ Wa + ba), k on partitions, 8 planes
                h = hpool.tile([128, 8, CH], hdt)
                for j in range(8):
                    psa = psapool.tile([128, CH], _F32)
                    nc.tensor.matmul(
                        psa[:], wa[:, j * 128:(j + 1) * 128],
                        ea[:, c * CH:(c + 1) * CH], start=True, stop=True,
                    )
                    nc.scalar.activation(
                        h[:, j, :], psa[:], _RELU, bias=ba[:, j:j + 1],
                    )

                for t in range(4):
                    tg = c * 4 + t            # global e-tile id
                    e0 = t * 128              # edge offset within chunk
                    # accumulator init: acc0 = xs @ B (theta-bias fold);
                    # lands in prod slots [32] and [33], each at half scale
                    psb = psbpool.tile([128, fo], _F32)
                    nc.tensor.matmul(
                        psb[:], xst[:, tg * 128:(tg + 1) * 128], bb[:],
                        start=True, stop=True,
                    )
                    prod = prpool.tile([128, fo, fi + 2], _BF16)
                    nc.scalar.activation(
                        prod[:, :, fi:fi + 2],
                        psb[:].unsqueeze(2).to_broadcast((128, fo, 2)),
                        _COPY, scale=0.5,
                    )

                    # stage B: theta tile [128 edges, d], bank-staggered
                    th = [thpool.tile([128, npb, fo], _F32, name="th", tag="th")
                          for _ in range(nb)]
                    for b in range(nb):
                        thf = th[b][:].rearrange("p i o -> p (i o)")
                        if USE_FP8:
                            for jp in range(4):
                                nc.tensor.matmul(
                                    thf,
                                    h[:, 2 * jp:2 * jp + 2, e0:e0 + 128],
                                    wb[:, 2 * jp:2 * jp + 2,
                                       b * 512:(b + 1) * 512],
                                    start=(jp == 0), stop=(jp == 3),
                                    perf_mode=_DR,
                                )
                        else:
                            for k in range(8):
                                nc.tensor.matmul(
                                    thf,
                                    h[:, k, e0:e0 + 128],
                                    wb[:, k, b * 512:(b + 1) * 512],
                                    start=(k == 0), stop=(k == 7),
                                )
                        # prod[:, o, i] = theta[e, i*fo+o] * xs[e, i]
                        nc.vector.tensor_tensor(
                            prod[:, :, b * npb:(b + 1) * npb],
                            th[b][:].rearrange("p i o -> p o i"),
                            xss[:, tg * fi + b * npb:tg * fi + (b + 1) * npb]
                            .unsqueeze(1).to_broadcast((128, fo, npb)),
                            _MUL,
                        )

                    # msg[e, o] = sum_i prod[e, o, i]  (incl. bias slots)
                    nc.vector.tensor_reduce(
                        msg[:, tg * fo:(tg + 1) * fo], prod[:],
                        mybir.AxisListType.X, _ADD,
                    )

            nc.sync.dma_start(msg_d[:], msg[:])

    nc.compile()
    return nc


def _get_nc(fo):
    if fo not in _NC_CACHE:
        _NC_CACHE[fo] = _build_pass(fo)
    return _NC_CACHE[fo]


def _relu(v):
    return np.maximum(v, 0.0)


class _SegMean:
    """Sort-based segment mean (np.add.at is too slow)."""

    def __init__(self, idx, n):
        self.n = n
        self.order = np.argsort(idx, kind="stable")
        sorted_idx = np.asarray(idx)[self.order]
        self.uniq, self.starts = np.unique(sorted_idx, return_index=True)
        self.cnt = np.maximum(
            np.bincount(np.asarray(idx), minlength=n), 1.0
        ).astype(np.float32)[:, None]

    def __call__(self, vals):
        out = np.zeros((self.n, vals.shape[1]), np.float32)
        out[self.uniq] = np.add.reduceat(vals[self.order], self.starts, axis=0)
        return out / self.cnt


def _pack_xs(xs_full, fi):
    """[E, fi] fp32 -> per-core ([128, NT*fi] fp32 tiled, [fi, EP] bf16 T)."""
    outs = []
    for i in range(NC):
        sh = np.zeros((EP, fi), np.float32)
        sh[:EPC] = xs_full[i * EPC:(i + 1) * EPC]
        tiled = np.ascontiguousarray(
            sh.reshape(NT, 128, fi).transpose(1, 0, 2).reshape(128, NT * fi))
        tr = np.ascontiguousarray(sh.T.astype(BF16))
        outs.append((tiled, tr))
    return outs


def _unpack_msg(results, fo):
    """per-core [128, NT*fo] -> [E, fo]."""
    parts = []
    for i in range(NC):
        m = results[i]["msg"]
        parts.append(
            m.reshape(128, NT, fo).transpose(1, 0, 2).reshape(EP, fo)[:EPC])
    return np.concatenate(parts, axis=0)


def _run_pass(fo, ea_bf_cores, xs_full, Wa, Wb, ba, Bb):
    nc = _get_nc(fo)
    wdt = F8E4 if USE_FP8 else BF16
    wb_packed = np.ascontiguousarray(
        np.concatenate([Wb[j * 128:(j + 1) * 128, :] for j in range(8)], axis=1)
    ).astype(wdt)
    wa_bf = np.ascontiguousarray(Wa).astype(BF16)
    ba_t = np.ascontiguousarray(ba.reshape(8, 128).T)
    bb_r = np.ascontiguousarray(Bb.reshape(32, fo)).astype(BF16)
    xs_packed = _pack_xs(xs_full, 32)
    in_maps = []
    for i in range(NC):
        in_maps.append(dict(
            eaT=ea_bf_cores[i], Wa=wa_bf, Wb=wb_packed, ba=ba_t, Bb=bb_r,
            xsS=xs_packed[i][0], xsT=xs_packed[i][1],
        ))
    res = bass_utils.run_bass_kernel_spmd(nc, in_maps, core_ids=list(range(NC)))
    LAST_RUNS.append(res)
    return _unpack_msg(res.results, fo)


def kernel(**inputs):
    x = np.asarray(inputs["x"], np.float32)
    edge_index = np.asarray(inputs["edge_index"])
    eap = np.asarray(inputs["edge_attr_packed"])
    batch = np.asarray(inputs["batch"])
    W1a = np.asarray(inputs["W1a"], np.float32)
    W1b = np.asarray(inputs["W1b"], np.float32)
    W2a = np.asarray(inputs["W2a"], np.float32)
    W2b = np.asarray(inputs["W2b"], np.float32)
    b1a = np.asarray(inputs["b1a"], np.float32)
    b1b = np.asarray(inputs["b1b"], np.float32)
    b2a = np.asarray(inputs["b2a"], np.float32)
    b2b = np.asarray(inputs["b2b"], np.float32)
    root1 = np.asarray(inputs["root1"], np.float32)
    bias1 = np.asarray(inputs["bias1"], np.float32)
    root2 = np.asarray(inputs["root2"], np.float32)
    bias2 = np.asarray(inputs["bias2"], np.float32)

    LAST_RUNS.clear()

    # MSB-first bit unpack -> [E, 16], per-core transposed bf16 copies
    shifts = np.arange(7, -1, -1, dtype=np.int32)
    ea = ((eap[:, :, None].astype(np.int32) >> shifts) & 1).reshape(E, -1)
    ea = ea.astype(np.float32)
    ea_cores = []
    for i in range(NC):
        sh = np.zeros((EP, EF), np.float32)
        sh[:EPC] = ea[i * EPC:(i + 1) * EPC]
        ea_cores.append(np.ascontiguousarray(sh.T.astype(BF16)))

    src, dst = edge_index[0], edge_index[1]
    segmean_dst = _SegMean(dst, N)

    msg1 = _run_pass(32, ea_cores, x[src], W1a, W1b, b1a, b1b)
    h = _relu(segmean_dst(msg1) + x @ root1 + bias1)

    msg2 = _run_pass(64, ea_cores, h[src], W2a, W2b, b2a, b2b)
    h = _relu(segmean_dst(msg2) + h @ root2 + bias2)

    g = _SegMean(batch, NG)(h)
    g = _relu(g @ np.asarray(inputs["fcW1"], np.float32) + np.asarray(inputs["fcb1"], np.float32))
    g = _relu(g @ np.asarray(inputs["fcW2"], np.float32) + np.asarray(inputs["fcb2"], np.float32))
    g = _relu(g @ np.asarray(inputs["fcW3"], np.float32) + np.asarray(inputs["fcb3"], np.float32))
    return (g @ np.asarray(inputs["fcW4"], np.float32) + np.asarray(inputs["fcb4"], np.float32)).astype(np.float32)


# revision 12
# speedup vs baseline: 6.4212x; 1.1639x over previous
"""GCN-with-edge-features kernel for 8 Trainium2 cores.

Strategy (per sharding hint): data-parallel over edges. The dominant
compute is the two edge-net MLPs producing per-edge weight matrices
  theta1 = relu(ea @ W1a + b1a) @ W1b + b1b   [E, 1024]
  theta2 = relu(ea @ W2a + b2a) @ W2b + b2b   [E, 2048]
(~630 GFLOP total), FUSED on-device with the per-edge matmul
msg[e,:] = x_src[e,:] @ theta[e].reshape(FI, FO) so the huge theta
intermediate never leaves PSUM. Layout: edges on the PSUM partition
dim, theta[e, d] with d free. The big K=1024 GEMMs run in fp8e4 with
DoubleRow perf mode (2 k-planes per matmul); the small stage-A GEMM is
bf16. The i-contraction runs on the vector engine as one broadcasted
tensor_tensor multiply per PSUM bank plus a single fused reduction;
the theta bias b folds into the reduction via acc0 = x_src @ B,
B[i, o] = b[i*FO+o], computed by a tiny bf16 matmul.

Two launches (layer 1, layer 2) with the segment-mean node aggregation
on host in between; graph pooling + FC head also on host (trivial).
12500 edges per core, padded to 12800 = 25 chunks of 512.
"""
import numpy as np

import sys
for p in ("/opt/trn_rl_repo",):
    if p not in sys.path:
        sys.path.append(p)

import ml_dtypes

from concourse import bass, bacc, mybir, tile
from concourse import bass_utils

E = 100000
N = 50000
NG = 2000
F_IN = 32
EF = 16
H = 32
H2 = 64
NC = 8
EPC = E // NC          # 12500 edges per core
CH = 512
NCHUNK = 25
EP = CH * NCHUNK       # 12800 padded edges per core
NT = EP // 128         # 100 e-tiles of 128 edges
D1 = H * F_IN          # 1024
D2 = H * H2            # 2048

USE_FP8 = True         # fp8e4 + DoubleRow for the big stage-B GEMMs

_F32 = mybir.dt.float32
_BF16 = mybir.dt.bfloat16
_F8 = mybir.dt.float8e4
_RELU = mybir.ActivationFunctionType.Relu
_COPY = mybir.ActivationFunctionType.Copy
_MUL = mybir.AluOpType.mult
_ADD = mybir.AluOpType.add
_DR = mybir.MatmulPerfMode.DoubleRow

_NC_CACHE = {}
LAST_RUNS = []  # BassKernelResults of the device launches in the last kernel() call

BF16 = ml_dtypes.bfloat16
F8E4 = ml_dtypes.float8_e4m3fn


def _build_pass(fo):
    """One GNN layer: fo = per-edge output width (32 for layer 1, 64 for
    layer 2). The per-edge input width is 32 for both layers."""
    fi = 32
    d = fi * fo                    # theta width: 1024 / 2048
    nb = d // 512                  # PSUM banks per e-tile: 2 / 4
    npb = 512 // fo                # i-values per bank: 16 / 8
    hdt = _F8 if USE_FP8 else _BF16

    nc = bacc.Bacc(None, target_bir_lowering=False)

    eaT_d = nc.dram_tensor("eaT", [EF, EP], _BF16, kind="ExternalInput")
    Wa_d = nc.dram_tensor("Wa", [EF, D1], _BF16, kind="ExternalInput")
    Wb_d = nc.dram_tensor("Wb", [128, 8 * d], hdt, kind="ExternalInput")
    ba_d = nc.dram_tensor("ba", [128, 8], _F32, kind="ExternalInput")
    Bb_d = nc.dram_tensor("Bb", [fi, fo], _BF16, kind="ExternalInput")
    xsS_d = nc.dram_tensor("xsS", [128, NT * fi], _BF16, kind="ExternalInput")
    xsT_d = nc.dram_tensor("xsT", [fi, EP], _BF16, kind="ExternalInput")
    msg_d = nc.dram_tensor("msg", [128, NT * fo], _BF16, kind="ExternalOutput")

    with tile.TileContext(nc) as tc:
        with (
            tc.tile_pool(name="w", bufs=1) as wpool,
            tc.tile_pool(name="h", bufs=2) as hpool,
            tc.tile_pool(name="pr", bufs=3) as prpool,
            tc.tile_pool(name="psa", bufs=2, space=bass.MemorySpace.PSUM) as psapool,
            tc.tile_pool(name="th", bufs=4, space=bass.MemorySpace.PSUM) as thpool,
            tc.tile_pool(name="psb", bufs=2, space=bass.MemorySpace.PSUM) as psbpool,
        ):
            wa = wpool.tile([EF, D1], _BF16)
            nc.sync.dma_start(wa[:], Wa_d[:])
            # [p, jp, bank, plane, n]: each DoubleRow moving slice
            # wb[:, jp, b, :, :] is 1024 contiguous bytes per partition
            wb = wpool.tile([128, 4, nb, 2, 512], hdt)
            nc.sync.dma_start(wb[:].rearrange("p a b c e -> p (a b c e)"), Wb_d[:])
            ba = wpool.tile([128, 8], _F32)
            nc.sync.dma_start(ba[:], ba_d[:])
            bb = wpool.tile([fi, fo], _BF16)
            nc.sync.dma_start(bb[:], Bb_d[:])
            ea = wpool.tile([EF, EP], _BF16)
            nc.sync.dma_start(ea[:], eaT_d[:])
            xss = wpool.tile([128, NT * fi], _BF16)
            nc.sync.dma_start(xss[:], xsS_d[:])
            xst = wpool.tile([fi, EP], _BF16)
            nc.sync.dma_start(xst[:], xsT_d[:])
            msg = wpool.tile([128, NT * fo], _BF16)

            def stage_a(c, h):
                # h = relu(ea @ Wa + ba), k on partitions, [jp, plane] layout
                for j in range(8):
                    psa = psapool.tile([128, CH], _F32, name="psa", tag="psa")
                    nc.tensor.matmul(
                        psa[:], wa[:, j * 128:(j + 1) * 128],
                        ea[:, c * CH:(c + 1) * CH], start=True, stop=True,
                    )
                    nc.scalar.activation(
                        h[:, j // 2, j % 2, :], psa[:], _RELU,
                        bias=ba[:, j:j + 1],
                    )

            hs = [None, None]
            hs[0] = hpool.tile([128, 4, 2, CH], hdt, name="h", tag="h")
            stage_a(0, hs[0])

            for c in range(NCHUNK):
                if c + 1 < NCHUNK:
                    # stage A for the NEXT chunk first, so its relus run on
                    # the scalar engine under this chunk's stage-B matmuls
                    hs[(c + 1) % 2] = hpool.tile(
                        [128, 4, 2, CH], hdt, name="h", tag="h")
                    stage_a(c + 1, hs[(c + 1) % 2])
                h = hs[c % 2]

                for t in range(4):
                    tg = c * 4 + t            # global e-tile id
                    e0 = t * 128              # edge offset within chunk
                    # accumulator init: acc0 = xs @ B (theta-bias fold);
                    # lands in prod slots [32] and [33], each at half scale
                    psb = psbpool.tile([128, fo], _F32)
                    nc.tensor.matmul(
                        psb[:], xst[:, tg * 128:(tg + 1) * 128], bb[:],
                        start=True, stop=True,
                    )
                    prod = prpool.tile([128, fo, fi + 2], _BF16)
                    nc.scalar.activation(
                        prod[:, :, fi:fi + 2],
                        psb[:].unsqueeze(2).to_broadcast((128, fo, 2)),
                        _COPY, scale=0.5,
                    )

                    # stage B: theta tile [128 edges, d], bank-staggered
                    th = [thpool.tile([128, npb, fo], _F32, name="th", tag="th")
                          for _ in range(nb)]
                    for b in range(nb):
                        thf = th[b][:].rearrange("p i o -> p (i o)")
                        if USE_FP8:
                            for jp in range(4):
                                nc.tensor.matmul(
                                    thf,
                                    h[:, jp, :, e0:e0 + 128],
                                    wb[:, jp, b, :, :],
                                    start=(jp == 0), stop=(jp == 3),
                                    perf_mode=_DR,
                                )
                        else:
                            for k in range(8):
                                nc.tensor.matmul(
                                    thf,
                                    h[:, k // 2, k % 2, e0:e0 + 128],
                                    wb[:, k // 2, b, k % 2, :],
                                    start=(k == 0), stop=(k == 7),
                                )
                        # prod[:, o, i] = theta[e, i*fo+o] * xs[e, i]
                        nc.vector.tensor_tensor(
                            prod[:, :, b * npb:(b + 1) * npb],
                            th[b][:].rearrange("p i o -> p o i"),
                            xss[:, tg * fi + b * npb:tg * fi + (b + 1) * npb]
                            .unsqueeze(1).to_broadcast((128, fo, npb)),
                            _MUL,
                        )

                    # msg[e, o] = sum_i prod[e, o, i]  (incl. bias slots);
                    # DVE accumulates fp32 internally, bf16 out only enables
                    # the 2x packed mode
                    with nc.allow_low_precision(reason="fp32 internal accum"):
                        nc.vector.tensor_reduce(
                            msg[:, tg * fo:(tg + 1) * fo], prod[:],
                            mybir.AxisListType.X, _ADD,
                        )

                if c % 5 == 4:
                    # stream out the finished 5-chunk slab
                    s = (c - 4) * 4 * fo
                    nc.sync.dma_start(
                        msg_d[:, s:(c + 1) * 4 * fo], msg[:, s:(c + 1) * 4 * fo])

    nc.compile()
    return nc


def _get_nc(fo):
    if fo not in _NC_CACHE:
        _NC_CACHE[fo] = _build_pass(fo)
    return _NC_CACHE[fo]


def _relu(v):
    return np.maximum(v, 0.0)


class _SegMean:
    """Sort-based segment mean (np.add.at is too slow)."""

    def __init__(self, idx, n):
        self.n = n
        self.order = np.argsort(idx, kind="stable")
        sorted_idx = np.asarray(idx)[self.order]
        self.uniq, self.starts = np.unique(sorted_idx, return_index=True)
        self.cnt = np.maximum(
            np.bincount(np.asarray(idx), minlength=n), 1.0
        ).astype(np.float32)[:, None]

    def __call__(self, vals):
        out = np.zeros((self.n, vals.shape[1]), np.float32)
        out[self.uniq] = np.add.reduceat(vals[self.order], self.starts, axis=0)
        return out / self.cnt


def _pack_xs(xs_full, fi):
    """[E, fi] fp32 -> per-core ([128, NT*fi] fp32 tiled, [fi, EP] bf16 T)."""
    outs = []
    for i in range(NC):
        sh = np.zeros((EP, fi), np.float32)
        sh[:EPC] = xs_full[i * EPC:(i + 1) * EPC]
        tiled = np.ascontiguousarray(
            sh.reshape(NT, 128, fi).transpose(1, 0, 2).reshape(128, NT * fi)
        ).astype(BF16)
        tr = np.ascontiguousarray(sh.T.astype(BF16))
        outs.append((tiled, tr))
    return outs


def _unpack_msg(results, fo):
    """per-core [128, NT*fo] -> [E, fo]."""
    parts = []
    for i in range(NC):
        m = np.asarray(results[i]["msg"]).astype(np.float32)
        parts.append(
            m.reshape(128, NT, fo).transpose(1, 0, 2).reshape(EP, fo)[:EPC])
    return np.concatenate(parts, axis=0)


def _run_pass(fo, ea_bf_cores, xs_full, Wa, Wb, ba, Bb):
    nc = _get_nc(fo)
    wdt = F8E4 if USE_FP8 else BF16
    d = Wb.shape[1]
    nb = d // 512
    # [k=1024, d] -> [p, jp, bank, plane, n] flattened; k = (2*jp+plane)*128+p
    wb_packed = np.ascontiguousarray(
        Wb.reshape(4, 2, 128, nb, 512).transpose(2, 0, 3, 1, 4).reshape(128, 8 * d)
    ).astype(wdt)
    wa_bf = np.ascontiguousarray(Wa).astype(BF16)
    ba_t = np.ascontiguousarray(ba.reshape(8, 128).T)
    bb_r = np.ascontiguousarray(Bb.reshape(32, fo)).astype(BF16)
    xs_packed = _pack_xs(xs_full, 32)
    in_maps = []
    for i in range(NC):
        in_maps.append(dict(
            eaT=ea_bf_cores[i], Wa=wa_bf, Wb=wb_packed, ba=ba_t, Bb=bb_r,
            xsS=xs_packed[i][0], xsT=xs_packed[i][1],
        ))
    res = bass_utils.run_bass_kernel_spmd(nc, in_maps, core_ids=list(range(NC)))
    LAST_RUNS.append(res)
    return _unpack_msg(res.results, fo)


def kernel(**inputs):
    x = np.asarray(inputs["x"], np.float32)
    edge_index = np.asarray(inputs["edge_index"])
    eap = np.asarray(inputs["edge_attr_packed"])
    batch = np.asarray(inputs["batch"])
    W1a = np.asarray(inputs["W1a"], np.float32)
    W1b = np.asarray(inputs["W1b"], np.float32)
    W2a = np.asarray(inputs["W2a"], np.float32)
    W2b = np.asarray(inputs["W2b"], np.float32)
    b1a = np.asarray(inputs["b1a"], np.float32)
    b1b = np.asarray(inputs["b1b"], np.float32)
    b2a = np.asarray(inputs["b2a"], np.float32)
    b2b = np.asarray(inputs["b2b"], np.float32)
    root1 = np.asarray(inputs["root1"], np.float32)
    bias1 = np.asarray(inputs["bias1"], np.float32)
    root2 = np.asarray(inputs["root2"], np.float32)
    bias2 = np.asarray(inputs["bias2"], np.float32)

    LAST_RUNS.clear()

    # MSB-first bit unpack -> [E, 16], per-core transposed bf16 copies
    shifts = np.arange(7, -1, -1, dtype=np.int32)
    ea = ((eap[:, :, None].astype(np.int32) >> shifts) & 1).reshape(E, -1)
    ea = ea.astype(np.float32)
    ea_cores = []
    for i in range(NC):
        sh = np.zeros((EP, EF), np.float32)
        sh[:EPC] = ea[i * EPC:(i + 1) * EPC]
        ea_cores.append(np.ascontiguousarray(sh.T.astype(BF16)))

    src, dst = edge_index[0], edge_index[1]
    segmean_dst = _SegMean(dst, N)

    msg1 = _run_pass(32, ea_cores, x[src], W1a, W1b, b1a, b1b)
    h = _relu(segmean_dst(msg1) + x @ root1 + bias1)

    msg2 = _run_pass(64, ea_cores, h[src], W2a, W2b, b2a, b2b)
    h = _relu(segmean_dst(msg2) + h @ root2 + bias2)

    g = _SegMean(batch, NG)(h)
    g = _relu(g @ np.asarray(inputs["fcW1"], np.float32) + np.asarray(inputs["fcb1"], np.float32))
    g = _relu(g @ np.asarray(inputs["fcW2"], np.float32) + np.asarray(inputs["fcb2"], np.float32))
    g = _relu(g @ np.asarray(inputs["fcW3"], np.float32) + np.asarray(inputs["fcb3"], np.float32))
    return (g @ np.asarray(inputs["fcW4"], np.float32) + np.asarray(inputs["fcb4"], np.float32)).astype(np.float32)


# revision 13
# speedup vs baseline: 6.4635x; 1.0066x over previous
"""GCN-with-edge-features kernel for 8 Trainium2 cores.

Strategy (per sharding hint): data-parallel over edges. The dominant
compute is the two edge-net MLPs producing per-edge weight matrices
  theta1 = relu(ea @ W1a + b1a) @ W1b + b1b   [E, 1024]
  theta2 = relu(ea @ W2a + b2a) @ W2b + b2b   [E, 2048]
(~630 GFLOP total), FUSED on-device with the per-edge matmul
msg[e,:] = x_src[e,:] @ theta[e].reshape(FI, FO) so the huge theta
intermediate never leaves PSUM. Layout: edges on the PSUM partition
dim, theta[e, d] with d free. The big K=1024 GEMMs run in fp8e4 with
DoubleRow perf mode (2 k-planes per matmul); the small stage-A GEMM is
bf16. The i-contraction runs on the vector engine as one broadcasted
tensor_tensor multiply per PSUM bank plus a single fused reduction;
the theta bias b folds into the reduction via acc0 = x_src @ B,
B[i, o] = b[i*FO+o], computed by a tiny bf16 matmul.

Two launches (layer 1, layer 2) with the segment-mean node aggregation
on host in between; graph pooling + FC head also on host (trivial).
12500 edges per core, padded to 12800 = 25 chunks of 512.
"""
import numpy as np

import sys
for p in ("/opt/trn_rl_repo",):
    if p not in sys.path:
        sys.path.append(p)

import ml_dtypes

from concourse import bass, bacc, mybir, tile
from concourse import bass_utils

E = 100000
N = 50000
NG = 2000
F_IN = 32
EF = 16
H = 32
H2 = 64
NC = 8
EPC = E // NC          # 12500 edges per core
CH = 512
NCHUNK = 25
EP = CH * NCHUNK       # 12800 padded edges per core
NT = EP // 128         # 100 e-tiles of 128 edges
D1 = H * F_IN          # 1024
D2 = H * H2            # 2048

USE_FP8 = True         # fp8e4 + DoubleRow for the big stage-B GEMMs

_F32 = mybir.dt.float32
_BF16 = mybir.dt.bfloat16
_F8 = mybir.dt.float8e4
_RELU = mybir.ActivationFunctionType.Relu
_COPY = mybir.ActivationFunctionType.Copy
_MUL = mybir.AluOpType.mult
_ADD = mybir.AluOpType.add
_DR = mybir.MatmulPerfMode.DoubleRow

_NC_CACHE = {}
LAST_RUNS = []  # BassKernelResults of the device launches in the last kernel() call

BF16 = ml_dtypes.bfloat16
F8E4 = ml_dtypes.float8_e4m3fn


def _build_pass(fo):
    """One GNN layer: fo = per-edge output width (32 for layer 1, 64 for
    layer 2). The per-edge input width is 32 for both layers."""
    fi = 32
    d = fi * fo                    # theta width: 1024 / 2048
    nb = d // 512                  # PSUM banks per e-tile: 2 / 4
    npb = 512 // fo                # i-values per bank: 16 / 8
    hdt = _F8 if USE_FP8 else _BF16

    nc = bacc.Bacc(None, target_bir_lowering=False)

    eaT_d = nc.dram_tensor("eaT", [EF, EP], _BF16, kind="ExternalInput")
    Wa_d = nc.dram_tensor("Wa", [EF, D1], _BF16, kind="ExternalInput")
    Wb_d = nc.dram_tensor("Wb", [128, 8 * d], hdt, kind="ExternalInput")
    ba_d = nc.dram_tensor("ba", [128, 8], _F32, kind="ExternalInput")
    Bb_d = nc.dram_tensor("Bb", [fi, fo], _BF16, kind="ExternalInput")
    xsS_d = nc.dram_tensor("xsS", [128, NT * fi], _BF16, kind="ExternalInput")
    xsT_d = nc.dram_tensor("xsT", [fi, EP], _BF16, kind="ExternalInput")
    msg_d = nc.dram_tensor("msg", [128, NT * fo], _BF16, kind="ExternalOutput")

    with tile.TileContext(nc) as tc:
        with (
            tc.tile_pool(name="w", bufs=1) as wpool,
            tc.tile_pool(name="h", bufs=2) as hpool,
            tc.tile_pool(name="pr", bufs=3) as prpool,
            tc.tile_pool(name="psa", bufs=2, space=bass.MemorySpace.PSUM) as psapool,
            tc.tile_pool(name="th", bufs=4, space=bass.MemorySpace.PSUM) as thpool,
            tc.tile_pool(name="psb", bufs=2, space=bass.MemorySpace.PSUM) as psbpool,
        ):
            wa = wpool.tile([EF, D1], _BF16)
            nc.sync.dma_start(wa[:], Wa_d[:])
            # [p, jp, bank, plane, n]: each DoubleRow moving slice
            # wb[:, jp, b, :, :] is 1024 contiguous bytes per partition
            wb = wpool.tile([128, 4, nb, 2, 512], hdt)
            nc.sync.dma_start(wb[:].rearrange("p a b c e -> p (a b c e)"), Wb_d[:])
            ba = wpool.tile([128, 8], _F32)
            nc.sync.dma_start(ba[:], ba_d[:])
            bb = wpool.tile([fi, fo], _BF16)
            nc.sync.dma_start(bb[:], Bb_d[:])
            ea = wpool.tile([EF, EP], _BF16)
            nc.sync.dma_start(ea[:], eaT_d[:])
            xss = wpool.tile([128, NT * fi], _BF16)
            nc.sync.dma_start(xss[:], xsS_d[:])
            xst = wpool.tile([fi, EP], _BF16)
            nc.sync.dma_start(xst[:], xsT_d[:])
            msg = wpool.tile([128, NT * fo], _BF16)

            def stage_a(c, h):
                # h = relu(ea @ Wa + ba), k on partitions, [jp, plane] layout
                for j in range(8):
                    psa = psapool.tile([128, CH], _F32, name="psa", tag="psa")
                    nc.tensor.matmul(
                        psa[:], wa[:, j * 128:(j + 1) * 128],
                        ea[:, c * CH:(c + 1) * CH], start=True, stop=True,
                    )
                    nc.scalar.activation(
                        h[:, j // 2, j % 2, :], psa[:], _RELU,
                        bias=ba[:, j:j + 1],
                    )

            hs = [None, None]
            hs[0] = hpool.tile([128, 4, 2, CH], hdt, name="h", tag="h")
            stage_a(0, hs[0])

            for c in range(NCHUNK):
                if c + 1 < NCHUNK:
                    # stage A for the NEXT chunk first, so its relus run on
                    # the scalar engine under this chunk's stage-B matmuls
                    hs[(c + 1) % 2] = hpool.tile(
                        [128, 4, 2, CH], hdt, name="h", tag="h")
                    stage_a(c + 1, hs[(c + 1) % 2])
                h = hs[c % 2]

                for t in range(4):
                    tg = c * 4 + t            # global e-tile id
                    e0 = t * 128              # edge offset within chunk
                    # accumulator init: acc0 = xs @ B (theta-bias fold);
                    # lands in prod slots [32] and [33], each at half scale
                    psb = psbpool.tile([128, fo], _F32)
                    nc.tensor.matmul(
                        psb[:], xst[:, tg * 128:(tg + 1) * 128], bb[:],
                        start=True, stop=True,
                    )
                    prod = prpool.tile([128, fo, fi + 2], _BF16)
                    nc.scalar.activation(
                        prod[:, :, fi:fi + 2],
                        psb[:].unsqueeze(2).to_broadcast((128, fo, 2)),
                        _COPY, scale=0.5,
                    )

                    # stage B: theta tile [128 edges, d]. Bank-pairs with
                    # jp-outer: consecutive matmuls share the stationary
                    # h-slice, letting the backend skip redundant LDWEIGHTS.
                    th = [thpool.tile([128, npb, fo], _F32, name="th", tag="th")
                          for _ in range(nb)]
                    thf = [t[:].rearrange("p i o -> p (i o)") for t in th]
                    for bp in range(nb // 2):
                        banks = (2 * bp, 2 * bp + 1)
                        if USE_FP8:
                            for jp in range(4):
                                for b in banks:
                                    nc.tensor.matmul(
                                        thf[b],
                                        h[:, jp, :, e0:e0 + 128],
                                        wb[:, jp, b, :, :],
                                        start=(jp == 0), stop=(jp == 3),
                                        perf_mode=_DR,
                                    )
                        else:
                            for k in range(8):
                                for b in banks:
                                    nc.tensor.matmul(
                                        thf[b],
                                        h[:, k // 2, k % 2, e0:e0 + 128],
                                        wb[:, k // 2, b, k % 2, :],
                                        start=(k == 0), stop=(k == 7),
                                    )
                        for b in banks:
                            # prod[:, o, i] = theta[e, i*fo+o] * xs[e, i]
                            nc.vector.tensor_tensor(
                                prod[:, :, b * npb:(b + 1) * npb],
                                th[b][:].rearrange("p i o -> p o i"),
                                xss[:, tg * fi + b * npb:tg * fi + (b + 1) * npb]
                                .unsqueeze(1).to_broadcast((128, fo, npb)),
                                _MUL,
                            )

                    # msg[e, o] = sum_i prod[e, o, i]  (incl. bias slots);
                    # DVE accumulates fp32 internally, bf16 out only enables
                    # the 2x packed mode
                    with nc.allow_low_precision(reason="fp32 internal accum"):
                        nc.vector.tensor_reduce(
                            msg[:, tg * fo:(tg + 1) * fo], prod[:],
                            mybir.AxisListType.X, _ADD,
                        )

                if c % 5 == 4:
                    # stream out the finished 5-chunk slab
                    s = (c - 4) * 4 * fo
                    nc.sync.dma_start(
                        msg_d[:, s:(c + 1) * 4 * fo], msg[:, s:(c + 1) * 4 * fo])

    nc.compile()
    return nc


def _get_nc(fo):
    if fo not in _NC_CACHE:
        _NC_CACHE[fo] = _build_pass(fo)
    return _NC_CACHE[fo]


def _relu(v):
    return np.maximum(v, 0.0)


class _SegMean:
    """Sort-based segment mean (np.add.at is too slow)."""

    def __init__(self, idx, n):
        self.n = n
        self.order = np.argsort(idx, kind="stable")
        sorted_idx = np.asarray(idx)[self.order]
        self.uniq, self.starts = np.unique(sorted_idx, return_index=True)
        self.cnt = np.maximum(
            np.bincount(np.asarray(idx), minlength=n), 1.0
        ).astype(np.float32)[:, None]

    def __call__(self, vals):
        out = np.zeros((self.n, vals.shape[1]), np.float32)
        out[self.uniq] = np.add.reduceat(vals[self.order], self.starts, axis=0)
        return out / self.cnt


def _pack_xs(xs_full, fi):
    """[E, fi] fp32 -> per-core ([128, NT*fi] fp32 tiled, [fi, EP] bf16 T)."""
    outs = []
    for i in range(NC):
        sh = np.zeros((EP, fi), np.float32)
        sh[:EPC] = xs_full[i * EPC:(i + 1) * EPC]
        tiled = np.ascontiguousarray(
            sh.reshape(NT, 128, fi).transpose(1, 0, 2).reshape(128, NT * fi)
        ).astype(BF16)
        tr = np.ascontiguousarray(sh.T.astype(BF16))
        outs.append((tiled, tr))
    return outs


def _unpack_msg(results, fo):
    """per-core [128, NT*fo] -> [E, fo]."""
    parts = []
    for i in range(NC):
        m = np.asarray(results[i]["msg"]).astype(np.float32)
        parts.append(
            m.reshape(128, NT, fo).transpose(1, 0, 2).reshape(EP, fo)[:EPC])
    return np.concatenate(parts, axis=0)


def _run_pass(fo, ea_bf_cores, xs_full, Wa, Wb, ba, Bb):
    nc = _get_nc(fo)
    wdt = F8E4 if USE_FP8 else BF16
    d = Wb.shape[1]
    nb = d // 512
    # [k=1024, d] -> [p, jp, bank, plane, n] flattened; k = (2*jp+plane)*128+p
    wb_packed = np.ascontiguousarray(
        Wb.reshape(4, 2, 128, nb, 512).transpose(2, 0, 3, 1, 4).reshape(128, 8 * d)
    ).astype(wdt)
    wa_bf = np.ascontiguousarray(Wa).astype(BF16)
    ba_t = np.ascontiguousarray(ba.reshape(8, 128).T)
    bb_r = np.ascontiguousarray(Bb.reshape(32, fo)).astype(BF16)
    xs_packed = _pack_xs(xs_full, 32)
    in_maps = []
    for i in range(NC):
        in_maps.append(dict(
            eaT=ea_bf_cores[i], Wa=wa_bf, Wb=wb_packed, ba=ba_t, Bb=bb_r,
            xsS=xs_packed[i][0], xsT=xs_packed[i][1],
        ))
    res = bass_utils.run_bass_kernel_spmd(nc, in_maps, core_ids=list(range(NC)))
    LAST_RUNS.append(res)
    return _unpack_msg(res.results, fo)


def kernel(**inputs):
    x = np.asarray(inputs["x"], np.float32)
    edge_index = np.asarray(inputs["edge_index"])
    eap = np.asarray(inputs["edge_attr_packed"])
    batch = np.asarray(inputs["batch"])
    W1a = np.asarray(inputs["W1a"], np.float32)
    W1b = np.asarray(inputs["W1b"], np.float32)
    W2a = np.asarray(inputs["W2a"], np.float32)
    W2b = np.asarray(inputs["W2b"], np.float32)
    b1a = np.asarray(inputs["b1a"], np.float32)
    b1b = np.asarray(inputs["b1b"], np.float32)
    b2a = np.asarray(inputs["b2a"], np.float32)
    b2b = np.asarray(inputs["b2b"], np.float32)
    root1 = np.asarray(inputs["root1"], np.float32)
    bias1 = np.asarray(inputs["bias1"], np.float32)
    root2 = np.asarray(inputs["root2"], np.float32)
    bias2 = np.asarray(inputs["bias2"], np.float32)

    LAST_RUNS.clear()

    # MSB-first bit unpack -> [E, 16], per-core transposed bf16 copies
    shifts = np.arange(7, -1, -1, dtype=np.int32)
    ea = ((eap[:, :, None].astype(np.int32) >> shifts) & 1).reshape(E, -1)
    ea = ea.astype(np.float32)
    ea_cores = []
    for i in range(NC):
        sh = np.zeros((EP, EF), np.float32)
        sh[:EPC] = ea[i * EPC:(i + 1) * EPC]
        ea_cores.append(np.ascontiguousarray(sh.T.astype(BF16)))

    src, dst = edge_index[0], edge_index[1]
    segmean_dst = _SegMean(dst, N)

    msg1 = _run_pass(32, ea_cores, x[src], W1a, W1b, b1a, b1b)
    h = _relu(segmean_dst(msg1) + x @ root1 + bias1)

    msg2 = _run_pass(64, ea_cores, h[src], W2a, W2b, b2a, b2b)
    h = _relu(segmean_dst(msg2) + h @ root2 + bias2)

    g = _SegMean(batch, NG)(h)
    g = _relu(g @ np.asarray(inputs["fcW1"], np.float32) + np.asarray(inputs["fcb1"], np.float32))
    g = _relu(g @ np.asarray(inputs["fcW2"], np.float32) + np.asarray(inputs["fcb2"], np.float32))
    g = _relu(g @ np.asarray(inputs["fcW3"], np.float32) + np.asarray(inputs["fcb3"], np.float32))
    return (g @ np.asarray(inputs["fcW4"], np.float32) + np.asarray(inputs["fcb4"], np.float32)).astype(np.float32)


# revision 15
# speedup vs baseline: 6.8779x; 1.0641x over previous
"""GCN-with-edge-features kernel for 8 Trainium2 cores.

Strategy (per sharding hint): data-parallel over edges. The dominant
compute is the two edge-net MLPs producing per-edge weight matrices
  theta1 = relu(ea @ W1a + b1a) @ W1b + b1b   [E, 1024]
  theta2 = relu(ea @ W2a + b2a) @ W2b + b2b   [E, 2048]
(~630 GFLOP total), FUSED on-device with the per-edge matmul
msg[e,:] = x_src[e,:] @ theta[e].reshape(FI, FO) so the huge theta
intermediate never leaves PSUM. Layout: edges on the PSUM partition
dim, theta[e, d] with d free. The big K=1024 GEMMs run in fp8e4 with
DoubleRow perf mode (2 k-planes per matmul); the small stage-A GEMM is
bf16. The i-contraction runs on the vector engine as one broadcasted
tensor_tensor multiply per PSUM bank plus a single fused reduction;
the theta bias b folds into the reduction via acc0 = x_src @ B,
B[i, o] = b[i*FO+o], computed by a tiny bf16 matmul.

Two launches (layer 1, layer 2) with the segment-mean node aggregation
on host in between; graph pooling + FC head also on host (trivial).
12500 edges per core, padded to 12800 = 25 chunks of 512.
"""
import numpy as np

import sys
for p in ("/opt/trn_rl_repo",):
    if p not in sys.path:
        sys.path.append(p)

import ml_dtypes

from concourse import bass, bacc, mybir, tile
from concourse import bass_utils

E = 100000
N = 50000
NG = 2000
F_IN = 32
EF = 16
H = 32
H2 = 64
NC = 8
EPC = E // NC          # 12500 edges per core
CH = 512
NCHUNK = 25
EP = CH * NCHUNK       # 12800 padded edges per core
NT = EP // 128         # 100 e-tiles of 128 edges
D1 = H * F_IN          # 1024
D2 = H * H2            # 2048

USE_FP8 = True         # fp8e4 + DoubleRow for the big stage-B GEMMs

_F32 = mybir.dt.float32
_BF16 = mybir.dt.bfloat16
_F8 = mybir.dt.float8e4
_RELU = mybir.ActivationFunctionType.Relu
_COPY = mybir.ActivationFunctionType.Copy
_MUL = mybir.AluOpType.mult
_ADD = mybir.AluOpType.add
_DR = mybir.MatmulPerfMode.DoubleRow

_NC_CACHE = {}
LAST_RUNS = []  # BassKernelResults of the device launches in the last kernel() call

BF16 = ml_dtypes.bfloat16
F8E4 = ml_dtypes.float8_e4m3fn


def _build_pass(fo):
    """One GNN layer: fo = per-edge output width (32 for layer 1, 64 for
    layer 2). The per-edge input width is 32 for both layers."""
    fi = 32
    d = fi * fo                    # theta width: 1024 / 2048
    nb = d // 512                  # PSUM banks per e-tile: 2 / 4
    npb = 512 // fo                # i-values per bank: 16 / 8
    hdt = _F8 if USE_FP8 else _BF16

    nc = bacc.Bacc(None, target_bir_lowering=False)

    eaT_d = nc.dram_tensor("eaT", [EF, EP], _BF16, kind="ExternalInput")
    Wa_d = nc.dram_tensor("Wa", [EF, D1], _BF16, kind="ExternalInput")
    Wb_d = nc.dram_tensor("Wb", [128, 8 * d], hdt, kind="ExternalInput")
    ba_d = nc.dram_tensor("ba", [128, 8], _F32, kind="ExternalInput")
    Bb_d = nc.dram_tensor("Bb", [fi, fo], _BF16, kind="ExternalInput")
    xsS_d = nc.dram_tensor("xsS", [128, NT * fi], _BF16, kind="ExternalInput")
    xsT_d = nc.dram_tensor("xsT", [fi, EP], _BF16, kind="ExternalInput")
    msg_d = nc.dram_tensor("msg", [128, NT * fo], _BF16, kind="ExternalOutput")

    with tile.TileContext(nc) as tc:
        with (
            tc.tile_pool(name="w", bufs=1) as wpool,
            tc.tile_pool(name="h", bufs=2) as hpool,
            tc.tile_pool(name="pr", bufs=3) as prpool,
            tc.tile_pool(name="psa", bufs=2, space=bass.MemorySpace.PSUM) as psapool,
            tc.tile_pool(name="th", bufs=4, space=bass.MemorySpace.PSUM) as thpool,
            tc.tile_pool(name="psb", bufs=2, space=bass.MemorySpace.PSUM) as psbpool,
        ):
            wa = wpool.tile([EF, D1], _BF16)
            nc.sync.dma_start(wa[:], Wa_d[:])
            # [p, jp, bank, plane, n]: each DoubleRow moving slice
            # wb[:, jp, b, :, :] is 1024 contiguous bytes per partition
            wb = wpool.tile([128, 4, nb, 2, 512], hdt)
            nc.sync.dma_start(wb[:].rearrange("p a b c e -> p (a b c e)"), Wb_d[:])
            ba = wpool.tile([128, 8], _F32)
            nc.sync.dma_start(ba[:], ba_d[:])
            bb = wpool.tile([fi, fo], _BF16)
            nc.sync.dma_start(bb[:], Bb_d[:])
            ea = wpool.tile([EF, EP], _BF16)
            nc.sync.dma_start(ea[:, :CH], eaT_d[:, :CH])  # chunk 0 lands first
            nc.sync.dma_start(ea[:, CH:], eaT_d[:, CH:])
            xss = wpool.tile([128, NT * fi], _BF16)
            nc.sync.dma_start(xss[:], xsS_d[:])
            xst = wpool.tile([fi, EP], _BF16)
            nc.sync.dma_start(xst[:], xsT_d[:])
            msg = wpool.tile([128, NT * fo], _BF16)

            def stage_a(c, h):
                # h = relu(ea @ Wa + ba), k on partitions, [jp, plane] layout
                for j in range(8):
                    psa = psapool.tile([128, CH], _F32, name="psa", tag="psa")
                    nc.tensor.matmul(
                        psa[:], wa[:, j * 128:(j + 1) * 128],
                        ea[:, c * CH:(c + 1) * CH], start=True, stop=True,
                    )
                    nc.scalar.activation(
                        h[:, j // 2, j % 2, :], psa[:], _RELU,
                        bias=ba[:, j:j + 1],
                    )

            hs = [None, None]
            hs[0] = hpool.tile([128, 4, 2, CH], hdt, name="h", tag="h")
            stage_a(0, hs[0])

            for c in range(NCHUNK):
                if c + 1 < NCHUNK:
                    # stage A for the NEXT chunk first, so its relus run on
                    # the scalar engine under this chunk's stage-B matmuls
                    hs[(c + 1) % 2] = hpool.tile(
                        [128, 4, 2, CH], hdt, name="h", tag="h")
                    stage_a(c + 1, hs[(c + 1) % 2])
                h = hs[c % 2]

                for t in range(4):
                    tg = c * 4 + t            # global e-tile id
                    e0 = t * 128              # edge offset within chunk
                    # accumulator init: acc0 = xs @ B (theta-bias fold);
                    # lands in prod slots [32] and [33], each at half scale
                    psb = psbpool.tile([128, fo], _F32)
                    nc.tensor.matmul(
                        psb[:], xst[:, tg * 128:(tg + 1) * 128], bb[:],
                        start=True, stop=True,
                    )
                    prod = prpool.tile([128, fo, fi + 2], _BF16)
                    nc.scalar.activation(
                        prod[:, :, fi:fi + 2],
                        psb[:].unsqueeze(2).to_broadcast((128, fo, 2)),
                        _COPY, scale=0.5,
                    )

                    # stage B: theta tile [128 edges, d]. Bank-pairs with
                    # jp-outer: consecutive matmuls share the stationary
                    # h-slice, letting the backend skip redundant LDWEIGHTS.
                    th = [thpool.tile([128, npb, fo], _F32, name="th", tag="th")
                          for _ in range(nb)]
                    thf = [t[:].rearrange("p i o -> p (i o)") for t in th]
                    for bp in range(nb // 2):
                        banks = (2 * bp, 2 * bp + 1)
                        if USE_FP8:
                            for jp in range(4):
                                for b in banks:
                                    nc.tensor.matmul(
                                        thf[b],
                                        h[:, jp, :, e0:e0 + 128],
                                        wb[:, jp, b, :, :],
                                        start=(jp == 0), stop=(jp == 3),
                                        perf_mode=_DR,
                                    )
                        else:
                            for k in range(8):
                                for b in banks:
                                    nc.tensor.matmul(
                                        thf[b],
                                        h[:, k // 2, k % 2, e0:e0 + 128],
                                        wb[:, k // 2, b, k % 2, :],
                                        start=(k == 0), stop=(k == 7),
                                    )
                        for b in banks:
                            # prod[:, o, i] = theta[e, i*fo+o] * xs[e, i]
                            nc.vector.tensor_tensor(
                                prod[:, :, b * npb:(b + 1) * npb],
                                th[b][:].rearrange("p i o -> p o i"),
                                xss[:, tg * fi + b * npb:tg * fi + (b + 1) * npb]
                                .unsqueeze(1).to_broadcast((128, fo, npb)),
                                _MUL,
                            )

                    # msg[e, o] = sum_i prod[e, o, i] (incl. bias slots).
                    # GpSimd (idle otherwise) folds the top half into the
                    # bottom half, halving the DVE reduction.
                    nc.gpsimd.tensor_tensor(
                        prod[:, :, 0:17], prod[:, :, 0:17],
                        prod[:, :, 17:34], _ADD,
                    )
                    with nc.allow_low_precision(reason="fp32 internal accum"):
                        nc.vector.tensor_reduce(
                            msg[:, tg * fo:(tg + 1) * fo], prod[:, :, 0:17],
                            mybir.AxisListType.X, _ADD,
                        )

                if c % 5 == 4:
                    # stream out the finished 5-chunk slab
                    s = (c - 4) * 4 * fo
                    nc.sync.dma_start(
                        msg_d[:, s:(c + 1) * 4 * fo], msg[:, s:(c + 1) * 4 * fo])

    nc.compile()
    return nc


def _get_nc(fo):
    if fo not in _NC_CACHE:
        _NC_CACHE[fo] = _build_pass(fo)
    return _NC_CACHE[fo]


def _relu(v):
    return np.maximum(v, 0.0)


class _SegMean:
    """Sort-based segment mean (np.add.at is too slow)."""

    def __init__(self, idx, n):
        self.n = n
        self.order = np.argsort(idx, kind="stable")
        sorted_idx = np.asarray(idx)[self.order]
        self.uniq, self.starts = np.unique(sorted_idx, return_index=True)
        self.cnt = np.maximum(
            np.bincount(np.asarray(idx), minlength=n), 1.0
        ).astype(np.float32)[:, None]

    def __call__(self, vals):
        out = np.zeros((self.n, vals.shape[1]), np.float32)
        out[self.uniq] = np.add.reduceat(vals[self.order], self.starts, axis=0)
        return out / self.cnt


def _pack_xs(xs_full, fi):
    """[E, fi] fp32 -> per-core ([128, NT*fi] fp32 tiled, [fi, EP] bf16 T)."""
    outs = []
    for i in range(NC):
        sh = np.zeros((EP, fi), np.float32)
        sh[:EPC] = xs_full[i * EPC:(i + 1) * EPC]
        tiled = np.ascontiguousarray(
            sh.reshape(NT, 128, fi).transpose(1, 0, 2).reshape(128, NT * fi)
        ).astype(BF16)
        tr = np.ascontiguousarray(sh.T.astype(BF16))
        outs.append((tiled, tr))
    return outs


def _unpack_msg(results, fo):
    """per-core [128, NT*fo] -> [E, fo]."""
    parts = []
    for i in range(NC):
        m = np.asarray(results[i]["msg"]).astype(np.float32)
        parts.append(
            m.reshape(128, NT, fo).transpose(1, 0, 2).reshape(EP, fo)[:EPC])
    return np.concatenate(parts, axis=0)


def _run_pass(fo, ea_bf_cores, xs_full, Wa, Wb, ba, Bb):
    nc = _get_nc(fo)
    wdt = F8E4 if USE_FP8 else BF16
    d = Wb.shape[1]
    nb = d // 512
    # [k=1024, d] -> [p, jp, bank, plane, n] flattened; k = (2*jp+plane)*128+p
    wb_packed = np.ascontiguousarray(
        Wb.reshape(4, 2, 128, nb, 512).transpose(2, 0, 3, 1, 4).reshape(128, 8 * d)
    ).astype(wdt)
    wa_bf = np.ascontiguousarray(Wa).astype(BF16)
    ba_t = np.ascontiguousarray(ba.reshape(8, 128).T)
    bb_r = np.ascontiguousarray(Bb.reshape(32, fo)).astype(BF16)
    xs_packed = _pack_xs(xs_full, 32)
    in_maps = []
    for i in range(NC):
        in_maps.append(dict(
            eaT=ea_bf_cores[i], Wa=wa_bf, Wb=wb_packed, ba=ba_t, Bb=bb_r,
            xsS=xs_packed[i][0], xsT=xs_packed[i][1],
        ))
    res = bass_utils.run_bass_kernel_spmd(nc, in_maps, core_ids=list(range(NC)))
    LAST_RUNS.append(res)
    return _unpack_msg(res.results, fo)


def kernel(**inputs):
    x = np.asarray(inputs["x"], np.float32)
    edge_index = np.asarray(inputs["edge_index"])
    eap = np.asarray(inputs["edge_attr_packed"])
    batch = np.asarray(inputs["batch"])
    W1a = np.asarray(inputs["W1a"], np.float32)
    W1b = np.asarray(inputs["W1b"], np.float32)
    W2a = np.asarray(inputs["W2a"], np.float32)
    W2b = np.asarray(inputs["W2b"], np.float32)
    b1a = np.asarray(inputs["b1a"], np.float32)
    b1b = np.asarray(inputs["b1b"], np.float32)
    b2a = np.asarray(inputs["b2a"], np.float32)
    b2b = np.asarray(inputs["b2b"], np.float32)
    root1 = np.asarray(inputs["root1"], np.float32)
    bias1 = np.asarray(inputs["bias1"], np.float32)
    root2 = np.asarray(inputs["root2"], np.float32)
    bias2 = np.asarray(inputs["bias2"], np.float32)

    LAST_RUNS.clear()

    # MSB-first bit unpack -> [E, 16], per-core transposed bf16 copies
    shifts = np.arange(7, -1, -1, dtype=np.int32)
    ea = ((eap[:, :, None].astype(np.int32) >> shifts) & 1).reshape(E, -1)
    ea = ea.astype(np.float32)
    ea_cores = []
    for i in range(NC):
        sh = np.zeros((EP, EF), np.float32)
        sh[:EPC] = ea[i * EPC:(i + 1) * EPC]
        ea_cores.append(np.ascontiguousarray(sh.T.astype(BF16)))

    src, dst = edge_index[0], edge_index[1]
    segmean_dst = _SegMean(dst, N)

    msg1 = _run_pass(32, ea_cores, x[src], W1a, W1b, b1a, b1b)
    h = _relu(segmean_dst(msg1) + x @ root1 + bias1)

    msg2 = _run_pass(64, ea_cores, h[src], W2a, W2b, b2a, b2b)
    h = _relu(segmean_dst(msg2) + h @ root2 + bias2)

    g = _SegMean(batch, NG)(h)
    g = _relu(g @ np.asarray(inputs["fcW1"], np.float32) + np.asarray(inputs["fcb1"], np.float32))
    g = _relu(g @ np.asarray(inputs["fcW2"], np.float32) + np.asarray(inputs["fcb2"], np.float32))
    g = _relu(g @ np.asarray(inputs["fcW3"], np.float32) + np.asarray(inputs["fcb3"], np.float32))
    return (g @ np.asarray(inputs["fcW4"], np.float32) + np.asarray(inputs["fcb4"], np.float32)).astype(np.float32)
